# revision 1
# baseline (speedup 1.0000x reference)
"""GQA attention kernel for 8 TRN2 NeuronCores.

Problem: x[4,2048,1024], 16 Q heads / 4 KV heads, head_dim 64 (torch-Linear
style projections, softmax(QK^T/8)V, output projection + bias).

Sharding: core c handles (batch b = c//2, half h2 = c%2) where a half is
2 KV heads = 8 Q heads = 512 hidden dims. Every core computes a partial
output projection over its 512 hidden dims; pairs (2b, 2b+1) AllReduce-add
their partials on-device, host reads the even core's buffer.

Per-core layouts (prepared on host, bf16):
  xt  [1024, 2048]  x[b]^T              (embed dim on partitions)
  wqt [1024, 512]   wq[512h2:512h2+512]^T
  wkt [1024, 128]   wk[128h2:128h2+128]^T
  wvt [1024, 128]   wv rows likewise
  wot [512, 1024]   wo^T rows for this half's hidden dims
  bo2 [1, 1024]     0.5 * bo (each pair member adds half -> sum = bo)

Inside: q^T/k^T computed in [dim, token] layout so QK^T needs no
transposes; S^T tiles [keys=128, queries=512] are exp'd on ACT straight
from PSUM; V is augmented with a ones column so the AV matmul also
produces the softmax denominators; normalization is deferred to the
attention output (gpsimd partition_broadcast of the reciprocal row).
"""

import sys
import numpy as np
from contextlib import ExitStack

sys.path.insert(0, "/opt/trn_rl_repo")

import ml_dtypes

from concourse import bass, tile, mybir


# ---------------------------------------------------------------------------
# This walrus build encodes at most 1-2 sync waits per instruction; the stock
# TileContext tail drain packs one wait per live proc onto a single Drain and
# fails codegen ("Too many sync wait commands"). Spread the waits over SP nop
# carriers instead.
def _patched_drain_and_barrier(self, tick_clock, wait_clock):
    from concourse.vector_clock import ScopedClock, VectorClock

    nc = self.nc
    gc = tick_clock.global_clock
    n = len(gc)
    for proc in range(n):
        t = gc[proc]
        if t <= 0:
            continue
        carrier = nc.sync.nop(nofuse=True)
        req = VectorClock([t if i == proc else 0 for i in range(n)])
        wait_clock.add_sem_waits(carrier.ins, ScopedClock({None: req}))
    nc.sync.drain()
    nc.all_engine_barrier()
    assert self.sems is not None
    popped = nc._tile_sem_poison_stack.pop()
    assert popped is self._sem_poison
    nc.clear_and_free_semaphores(list(self.sems.allocated().values()))
    nc.all_engine_barrier()


tile.TileContext._drain_and_barrier = _patched_drain_and_barrier


def _split_excess_waits(nc, max_waits=1):
    """Hoist all but one sync wait per instruction onto dedicated
    EventSemaphore carriers placed immediately before it on the same engine
    (same blocking semantics, one wait per encoded instruction)."""
    n_new = 0
    for bb in nc.main_func.blocks:
        il = list(bb.instructions)
        out = []
        changed = False
        for ins in il:
            si = ins.sync_info
            if si is not None:
                w = list(si.on_wait)
                if len(w) > max_waits:
                    for extra in w[max_waits:]:
                        ev = mybir.InstEventSemaphore(
                            name=f"{ins.name}-wsp{n_new}", engine=ins.engine)
                        n_new += 1
                        ev.sync_info = type(si)(on_wait=[extra], on_update=[])
                        nc.register_instruction(ev, overwrite=True)
                        out.append(ev)
                    si.on_wait = w[:max_waits]
                    changed = True
            out.append(ins)
        if changed:
            bb.instructions = out
# ---------------------------------------------------------------------------

B, N, D = 4, 2048, 1024
DH = 64  # head dim
HID = 512  # hidden dims per core (8 q heads)
NCORES = 8
P = 128
SCALE = DH ** -0.5
BF16 = mybir.dt.bfloat16
F32 = mybir.dt.float32

NB = N // P  # 16 token blocks of 128
NK = D // P  # 8 contraction chunks of 128
NQB = 4  # n blocks of 512 for attention moving dim
VW = 256  # v chunk width: [64 v_h0 | 64 ones | 64 v_h1 | 64 ones]


def build_nc(st_bufs=3, pt_bufs=4, trace_friendly=False):
    nc = bass.Bass(target_bir_lowering=False, debug=False, num_devices=NCORES)

    xt = nc.declare_dram_parameter("xt", [D, N], BF16, isOutput=False)
    wqt = nc.declare_dram_parameter("wqt", [D, HID], BF16, isOutput=False)
    wkt = nc.declare_dram_parameter("wkt", [D, P], BF16, isOutput=False)
    wvt = nc.declare_dram_parameter("wvt", [D, P], BF16, isOutput=False)
    wot = nc.declare_dram_parameter("wot", [HID, D], BF16, isOutput=False)
    bo2 = nc.declare_dram_parameter("bo2", [1, D], F32, isOutput=False)
    out_p = nc.declare_dram_parameter("out_p", [N, D], F32, isOutput=True)

    ob = nc.dram_tensor("ob", [N, D], F32)  # partial o-proj (collective in)
    rb = nc.dram_tensor("rb", [N, D], F32)  # pair-summed (collective out)

    with tile.TileContext(nc) as tc, ExitStack() as ctx:
        const = ctx.enter_context(tc.tile_pool(name="const", bufs=1))
        work = ctx.enter_context(tc.tile_pool(name="work", bufs=1))
        ppool = ctx.enter_context(tc.tile_pool(name="ppool", bufs=3, space="PSUM"))
        stpool = ctx.enter_context(tc.tile_pool(name="stp", bufs=st_bufs, space="PSUM"))
        avpool = ctx.enter_context(tc.tile_pool(name="avp", bufs=2, space="PSUM"))
        ptpool = ctx.enter_context(tc.tile_pool(name="ptp", bufs=pt_bufs))
        smallp = ctx.enter_context(tc.tile_pool(name="smallp", bufs=3))
        outp = ctx.enter_context(tc.tile_pool(name="outp", bufs=3))

        # ---- load inputs -------------------------------------------------
        xt_sb = const.tile([P, NK * N], BF16)
        for kc in range(NK):
            nc.sync.dma_start(out=xt_sb[:, kc * N:(kc + 1) * N],
                              in_=xt[kc * P:(kc + 1) * P, :])
        wqt_sb = const.tile([P, NK * HID], BF16)
        wkt_sb = const.tile([P, NK * P], BF16)
        wvt_sb = const.tile([P, NK * P], BF16)
        wot_sb = const.tile([P, 4 * D], BF16)
        for kc in range(NK):
            nc.sync.dma_start(out=wqt_sb[:, kc * HID:(kc + 1) * HID],
                              in_=wqt[kc * P:(kc + 1) * P, :])
            nc.sync.dma_start(out=wkt_sb[:, kc * P:(kc + 1) * P],
                              in_=wkt[kc * P:(kc + 1) * P, :])
            nc.sync.dma_start(out=wvt_sb[:, kc * P:(kc + 1) * P],
                              in_=wvt[kc * P:(kc + 1) * P, :])
        for ic in range(4):
            nc.sync.dma_start(out=wot_sb[:, ic * D:(ic + 1) * D],
                              in_=wot[ic * P:(ic + 1) * P, :])
        bo_row = const.tile([1, D], F32)
        nc.sync.dma_start(out=bo_row[:], in_=bo2[0:1, :])
        ones_row = const.tile([1, P], F32)
        nc.vector.memset(ones_row[:], 1.0)
        # partition-broadcast via PE outer product (gpsimd InstISA is not
        # supported by this walrus build)
        bo_bc = const.tile([P, D], F32)
        for jh in range(2):
            bps = ppool.tile([P, 512], F32, tag="proj")
            nc.tensor.matmul(bps[:], lhsT=ones_row[:, 0:P],
                             rhs=bo_row[:, jh * 512:(jh + 1) * 512],
                             start=True, stop=True)
            nc.vector.tensor_copy(bo_bc[:, jh * 512:(jh + 1) * 512], bps[:])

        # ---- projections -------------------------------------------------
        # q^T [512, 2048] as 4 partition-blocks; k^T [128, 2048]; v natural.
        qt_sb = work.tile([P, 4 * N], BF16, tag="qt")
        kt_sb = work.tile([P, N], BF16, tag="kt")
        v_sb = work.tile([P, NB * VW], BF16, tag="v")
        nc.vector.memset(v_sb[:], 1.0)  # ones columns survive the copies

        for mb in range(4):  # q output-dim blocks
            for nb in range(NQB):
                ps = ppool.tile([P, 512], F32, tag="proj")
                for kc in range(NK):
                    nc.tensor.matmul(
                        ps[:],
                        lhsT=wqt_sb[:, kc * HID + mb * P: kc * HID + (mb + 1) * P],
                        rhs=xt_sb[:, kc * N + nb * 512: kc * N + (nb + 1) * 512],
                        start=(kc == 0), stop=(kc == NK - 1),
                    )
                nc.vector.tensor_copy(
                    qt_sb[:, mb * N + nb * 512: mb * N + (nb + 1) * 512], ps[:])

        for nb in range(NQB):
            ps = ppool.tile([P, 512], F32, tag="proj")
            for kc in range(NK):
                nc.tensor.matmul(
                    ps[:],
                    lhsT=wkt_sb[:, kc * P:(kc + 1) * P],
                    rhs=xt_sb[:, kc * N + nb * 512: kc * N + (nb + 1) * 512],
                    start=(kc == 0), stop=(kc == NK - 1),
                )
            nc.vector.tensor_copy(kt_sb[:, nb * 512:(nb + 1) * 512], ps[:])

        for mb in range(NB):  # v in natural [token, dim] layout
            ps = ppool.tile([P, P], F32, tag="proj")
            for kc in range(NK):
                nc.tensor.matmul(
                    ps[:],
                    lhsT=xt_sb[:, kc * N + mb * P: kc * N + (mb + 1) * P],
                    rhs=wvt_sb[:, kc * P:(kc + 1) * P],
                    start=(kc == 0), stop=(kc == NK - 1),
                )
            nc.vector.tensor_copy(v_sb[:, mb * VW: mb * VW + 64], ps[:, 0:64])
            nc.vector.tensor_copy(v_sb[:, mb * VW + 128: mb * VW + 192], ps[:, 64:128])

        # ---- attention ---------------------------------------------------
        # hidden^T [512, 2048] bf16, normalized attention outputs
        hid_sb = work.tile([P, 4 * N], BF16, tag="hid")

        # head h lives in q/hid block h%4 at partition half h//4 == its kv
        # head's half in kt (wqt cols / wot rows are host-reordered to match),
        # so the QK matmul's lhsT and rhs share a base partition.
        for h in range(8):  # local q heads
            kv = h // 4  # local kv head
            qp = 64 * kv  # partition offset in qt block
            qb = h % 4  # qt partition-block
            hp = qp
            hb = qb
            for nb in range(NQB):
                # av rows 0:64 = unnormalized attention out (v columns);
                # rows 64:128 = softmax denominators, broadcast across 64
                # partitions for free by the ones columns of v_aug.
                av = avpool.tile([P, 512], F32, tag="av")
                for mc in range(NB):
                    st = stpool.tile([P, 512], F32, tag="st")
                    nc.tensor.matmul(
                        st[:],
                        lhsT=kt_sb[64 * kv:64 * kv + 64, mc * P:(mc + 1) * P],
                        rhs=qt_sb[qp:qp + 64, qb * N + nb * 512: qb * N + (nb + 1) * 512],
                        start=True, stop=True,
                    )
                    pt = ptpool.tile([P, 512], BF16, tag="pt")
                    nc.scalar.activation(pt[:], st[:],
                                         mybir.ActivationFunctionType.Exp,
                                         scale=SCALE)
                    nc.tensor.matmul(
                        av[:],
                        lhsT=v_sb[:, mc * VW + 128 * kv: mc * VW + 128 * kv + 128],
                        rhs=pt[:],
                        start=(mc == 0), stop=(mc == NB - 1),
                    )
                den = smallp.tile([64, 512], F32, tag="den")
                nc.vector.reciprocal(den[:], av[64:128, :])
                nc.vector.tensor_tensor(
                    out=hid_sb[hp:hp + 64, hb * N + nb * 512: hb * N + (nb + 1) * 512],
                    in0=av[0:64, :], in1=den[:],
                    op=mybir.AluOpType.mult,
                )

        # ---- output projection ------------------------------------------
        for tb in range(NB):
            ot = outp.tile([P, D], F32, tag="osb")
            for jh in range(2):
                ps = ppool.tile([P, 512], F32, tag="proj")
                for ic in range(4):
                    nc.tensor.matmul(
                        ps[:],
                        lhsT=hid_sb[:, ic * N + tb * P: ic * N + (tb + 1) * P],
                        rhs=wot_sb[:, ic * D + jh * 512: ic * D + (jh + 1) * 512],
                        start=(ic == 0), stop=(ic == 3),
                    )
                nc.vector.tensor_tensor(
                    out=ot[:, jh * 512:(jh + 1) * 512],
                    in0=ps[:], in1=bo_bc[:, jh * 512:(jh + 1) * 512],
                    op=mybir.AluOpType.add,
                )
            nc.sync.dma_start(out=ob[tb * P:(tb + 1) * P, :], in_=ot[:])

        # ---- pair all-reduce + output -----------------------------------
        nc.gpsimd.collective_compute(
            "AllReduce", mybir.AluOpType.add,
            replica_groups=[[0, 1], [2, 3], [4, 5], [6, 7]],
            ins=[ob.ap().opt()], outs=[rb.ap().opt()],
        )
        for tb in range(4):
            nc.sync.dma_start(out=out_p[tb * 512:(tb + 1) * 512, :],
                              in_=rb[tb * 512:(tb + 1) * 512, :])

    _split_excess_waits(nc)
    return nc


def make_in_maps(x, wq, wk, wv, wo, bo):
    bf = ml_dtypes.bfloat16
    # local head h -> device slot (block h%4, half h//4): permuted head order
    hperm = [0, 4, 1, 5, 2, 6, 3, 7]
    dperm = np.concatenate([np.arange(64 * h, 64 * h + 64) for h in hperm])
    in_maps = []
    for c in range(NCORES):
        b, h2 = c // 2, c % 2
        wq_c = wq[HID * h2:HID * (h2 + 1)][dperm]  # [512, 1024] permuted rows
        wot_c = wo.T[HID * h2:HID * (h2 + 1)][dperm]  # [512, 1024] same perm
        in_maps.append({
            "xt": np.ascontiguousarray(x[b].T).astype(bf),
            "wqt": np.ascontiguousarray(wq_c.T).astype(bf),
            "wkt": np.ascontiguousarray(wk[P * h2:P * (h2 + 1)].T).astype(bf),
            "wvt": np.ascontiguousarray(wv[P * h2:P * (h2 + 1)].T).astype(bf),
            "wot": np.ascontiguousarray(wot_c).astype(bf),
            "bo2": (0.5 * bo).astype(np.float32).reshape(1, D),
        })
    return in_maps


_CACHED_NC = None


def kernel(x, wq, wk, wv, wo, bo, _trace=False, _trace_kwargs=None):
    global _CACHED_NC
    from concourse.bass_utils import run_bass_kernel_spmd

    if _CACHED_NC is None:
        _CACHED_NC = build_nc()
    nc = _CACHED_NC

    in_maps = make_in_maps(
        np.asarray(x, np.float32), np.asarray(wq, np.float32),
        np.asarray(wk, np.float32), np.asarray(wv, np.float32),
        np.asarray(wo, np.float32), np.asarray(bo, np.float32))

    res = run_bass_kernel_spmd(
        nc, in_maps, core_ids=list(range(NCORES)),
        trace=_trace, **(_trace_kwargs or {}))

    out = np.empty((B, N, D), np.float32)
    for b in range(B):
        out[b] = res.results[2 * b]["out_p"]
    if _trace:
        kernel._last_results = res
    return out



# revision 5
# speedup vs baseline: 1.3422x; 1.3422x over previous
"""GQA attention kernel for 8 TRN2 NeuronCores — sequence-split edition.

Problem: x[4,2048,1024], 16 Q heads / 4 KV heads, head_dim 64 (torch-Linear
style projections, softmax(QK^T/8)V, output projection + bias).

Sharding: core c handles (batch b = c//2, query-half qh = c%2): it computes
the FULL output rows for its 1024 query tokens (all 16 heads + o-proj), so
there is NO collective — each core DMAs its own [1024, 1024] f32 slab out.
K/V are computed for all 2048 keys on both cores of a pair (cheap).

The host permutes each core's token order so its own queries come first
(attention is key-order invariant), which keeps the SPMD program uniform.
Q-head order is permuted on the host so each head's 64 q-dims sit at the
same partition offset as its KV head's k-dims (QK lhsT/rhs share a base
partition): device q block j (0..7), offset o in {0,64} holds head
(kv = 2*(j//4) + o//64, g = j%4); wo^T rows are permuted identically.

Inside: q^T/k^T in [dim, token] layout so QK needs no transposes; S^T tiles
[keys=128, queries=512] are exp'd on ACT straight from PSUM; V is augmented
with 64 ones columns so the AV matmul also produces the softmax denominators
on partitions 64:128; normalization = reciprocal_approx_fast (single fast
custom-DVE op) + one DVE multiply writing hid^T in bf16. O-proj + bias-add +
output DMA stream per 512-query block, overlapped with the next block's
attention (ACT exp is the critical engine; everything else hides under it).
"""

import sys
import numpy as np
from contextlib import ExitStack

sys.path.insert(0, "/opt/trn_rl_repo")

import ml_dtypes

from concourse import bass, tile, mybir


# ---------------------------------------------------------------------------
# This walrus build encodes at most 1-2 sync waits per instruction; the stock
# TileContext tail drain packs one wait per live proc onto a single Drain and
# fails codegen ("Too many sync wait commands"). Spread the waits over SP nop
# carriers instead.
def _patched_drain_and_barrier(self, tick_clock, wait_clock):
    from concourse.vector_clock import ScopedClock, VectorClock

    nc = self.nc
    gc = tick_clock.global_clock
    n = len(gc)
    for proc in range(n):
        t = gc[proc]
        if t <= 0:
            continue
        carrier = nc.sync.nop(nofuse=True)
        req = VectorClock([t if i == proc else 0 for i in range(n)])
        wait_clock.add_sem_waits(carrier.ins, ScopedClock({None: req}))
    nc.sync.drain()
    nc.all_engine_barrier()
    assert self.sems is not None
    popped = nc._tile_sem_poison_stack.pop()
    assert popped is self._sem_poison
    nc.clear_and_free_semaphores(list(self.sems.allocated().values()))
    nc.all_engine_barrier()


tile.TileContext._drain_and_barrier = _patched_drain_and_barrier


def _split_excess_waits(nc, max_waits=1):
    """Hoist all but one sync wait per instruction onto dedicated
    EventSemaphore carriers placed immediately before it on the same engine
    (same blocking semantics, one wait per encoded instruction)."""
    n_new = 0
    for bb in nc.main_func.blocks:
        il = list(bb.instructions)
        out = []
        changed = False
        for ins in il:
            si = ins.sync_info
            if si is not None:
                w = list(si.on_wait)
                if len(w) > max_waits:
                    for extra in w[max_waits:]:
                        ev = mybir.InstEventSemaphore(
                            name=f"{ins.name}-wsp{n_new}", engine=ins.engine)
                        n_new += 1
                        ev.sync_info = type(si)(on_wait=[extra], on_update=[])
                        nc.register_instruction(ev, overwrite=True)
                        out.append(ev)
                    si.on_wait = w[:max_waits]
                    changed = True
            out.append(ins)
        if changed:
            bb.instructions = out
# ---------------------------------------------------------------------------

B, N, D = 4, 2048, 1024
DH = 64          # head dim
NQ = 1024        # queries per core
NCORES = 8
P = 128
SCALE = DH ** -0.5
BF16 = mybir.dt.bfloat16
F32 = mybir.dt.float32

NKB = N // P     # 16 key blocks of 128
NKC = D // P     # 8 contraction chunks of 128
KVD = 256        # total kv dims
VW = 512         # v chunk width per key block: 4 x [64 v | 64 ones]


def build_nc(st_bufs=2, av_bufs=2, pt_bufs=4):
    nc = bass.Bass(target_bir_lowering=False, debug=False, num_devices=NCORES)

    xt = nc.declare_dram_parameter("xt", [D, N], BF16, isOutput=False)
    wqt = nc.declare_dram_parameter("wqt", [D, D], BF16, isOutput=False)
    wkt = nc.declare_dram_parameter("wkt", [D, KVD], BF16, isOutput=False)
    wvt = nc.declare_dram_parameter("wvt", [D, KVD], BF16, isOutput=False)
    wot = nc.declare_dram_parameter("wot", [D, D], BF16, isOutput=False)
    bo_in = nc.declare_dram_parameter("bo_in", [1, D], F32, isOutput=False)
    out_p = nc.declare_dram_parameter("out_p", [NQ, D], F32, isOutput=True)

    with tile.TileContext(nc) as tc, ExitStack() as ctx:
        const = ctx.enter_context(tc.tile_pool(name="const", bufs=1))
        work = ctx.enter_context(tc.tile_pool(name="work", bufs=1))
        ppool = ctx.enter_context(tc.tile_pool(name="ppool", bufs=2, space="PSUM"))
        stpool = ctx.enter_context(tc.tile_pool(name="stp", bufs=st_bufs, space="PSUM"))
        avpool = ctx.enter_context(tc.tile_pool(name="avp", bufs=av_bufs, space="PSUM"))
        ptpool = ctx.enter_context(tc.tile_pool(name="ptp", bufs=pt_bufs))
        smallp = ctx.enter_context(tc.tile_pool(name="smallp", bufs=3))
        outp = ctx.enter_context(tc.tile_pool(name="outp", bufs=3))

        # ---- load inputs -------------------------------------------------
        xt_sb = const.tile([P, NKC * N], BF16)
        wkt_sb = const.tile([P, NKC * KVD], BF16)
        wvt_sb = const.tile([P, NKC * KVD], BF16)
        wqt_sb = const.tile([P, NKC * D], BF16)
        wot_sb = const.tile([P, NKC * D], BF16)
        for kc in range(NKC):
            nc.sync.dma_start(out=xt_sb[:, kc * N:(kc + 1) * N],
                              in_=xt[kc * P:(kc + 1) * P, :])
            nc.sync.dma_start(out=wkt_sb[:, kc * KVD:(kc + 1) * KVD],
                              in_=wkt[kc * P:(kc + 1) * P, :])
            nc.sync.dma_start(out=wvt_sb[:, kc * KVD:(kc + 1) * KVD],
                              in_=wvt[kc * P:(kc + 1) * P, :])
            nc.sync.dma_start(out=wqt_sb[:, kc * D:(kc + 1) * D],
                              in_=wqt[kc * P:(kc + 1) * P, :])
            nc.sync.dma_start(out=wot_sb[:, kc * D:(kc + 1) * D],
                              in_=wot[kc * P:(kc + 1) * P, :])
        bo_row = const.tile([1, D], F32)
        nc.sync.dma_start(out=bo_row[:], in_=bo_in[0:1, :])
        ones_row = const.tile([1, P], F32)
        nc.vector.memset(ones_row[:], 1.0)
        # bias partition-broadcast via PE outer product
        bo_bc = const.tile([P, D], F32)
        for jh in range(2):
            bps = ppool.tile([P, 512], F32, tag="proj")
            nc.tensor.matmul(bps[:], lhsT=ones_row[:, 0:P],
                             rhs=bo_row[:, jh * 512:(jh + 1) * 512],
                             start=True, stop=True)
            nc.vector.tensor_copy(bo_bc[:, jh * 512:(jh + 1) * 512], bps[:])

        # ---- projections -------------------------------------------------
        # k^T [256, 2048] as 2 partition-blocks (kv head kv at block kv//2,
        # partition offset (kv%2)*64)
        kt_sb = work.tile([P, 2 * N], BF16, tag="kt")
        for m2 in range(2):
            for nb in range(4):
                ps = ppool.tile([P, 512], F32, tag="proj")
                for kc in range(NKC):
                    nc.tensor.matmul(
                        ps[:],
                        lhsT=wkt_sb[:, kc * KVD + m2 * P: kc * KVD + (m2 + 1) * P],
                        rhs=xt_sb[:, kc * N + nb * 512: kc * N + (nb + 1) * 512],
                        start=(kc == 0), stop=(kc == NKC - 1),
                    )
                nc.vector.tensor_copy(kt_sb[:, m2 * N + nb * 512: m2 * N + (nb + 1) * 512], ps[:])

        # v natural [keys, vdim], augmented: per key block 4 x [64 v | 64 ones]
        v_sb = work.tile([P, NKB * VW], BF16, tag="v")
        nc.vector.memset(v_sb[:], 1.0)  # ones columns survive the copies
        for kb in range(NKB):
            ps = ppool.tile([P, KVD], F32, tag="proj")
            for kc in range(NKC):
                nc.tensor.matmul(
                    ps[:],
                    lhsT=xt_sb[:, kc * N + kb * P: kc * N + (kb + 1) * P],
                    rhs=wvt_sb[:, kc * KVD:(kc + 1) * KVD],
                    start=(kc == 0), stop=(kc == NKC - 1),
                )
            for kv in range(4):
                nc.vector.tensor_copy(
                    v_sb[:, kb * VW + kv * P: kb * VW + kv * P + 64],
                    ps[:, kv * 64:(kv + 1) * 64])

        # q^T [1024, 1024] in device head order, 8 partition-block tiles
        qt = []
        for j in range(8):
            qt_j = work.tile([P, NQ], BF16, tag=f"qt{j}")
            qt.append(qt_j)
            for q2 in range(2):
                ps = ppool.tile([P, 512], F32, tag="proj")
                for kc in range(NKC):
                    nc.tensor.matmul(
                        ps[:],
                        lhsT=wqt_sb[:, kc * D + j * P: kc * D + (j + 1) * P],
                        rhs=xt_sb[:, kc * N + q2 * 512: kc * N + (q2 + 1) * 512],
                        start=(kc == 0), stop=(kc == NKC - 1),
                    )
                nc.vector.tensor_copy(qt_j[:, q2 * 512:(q2 + 1) * 512], ps[:])

        # hidden^T [1024, 1024] bf16, device head order (matches wot rows)
        hid = []
        for j in range(8):
            hid_j = work.tile([P, NQ], BF16, tag=f"hid{j}")
            hid.append(hid_j)

        # ---- attention + streamed o-proj ---------------------------------
        for qb in range(2):  # 512-query blocks
            for j in range(8):
                for o in (0, 64):
                    kv = 2 * (j // 4) + o // 64
                    av = avpool.tile([P, 512], F32, tag="av")
                    for kb2 in range(NKB // 2):  # key blocks in fused pairs
                        st = stpool.tile([P, 1024], F32, tag="st")  # 2 banks
                        for u in range(2):
                            kb = 2 * kb2 + u
                            nc.tensor.matmul(
                                st[:, u * 512:(u + 1) * 512],
                                lhsT=kt_sb[o:o + 64, (j // 4) * N + kb * P: (j // 4) * N + (kb + 1) * P],
                                rhs=qt[j][o:o + 64, qb * 512:(qb + 1) * 512],
                                start=True, stop=True,
                            )
                        # one ACT pass over both key blocks amortizes the
                        # ~290ns ACTIVATE pipeline overhead
                        pt = ptpool.tile([P, 1024], BF16, tag="pt")
                        nc.scalar.activation(pt[:], st[:],
                                             mybir.ActivationFunctionType.Exp,
                                             scale=SCALE)
                        for u in range(2):
                            kb = 2 * kb2 + u
                            nc.tensor.matmul(
                                av[:],
                                lhsT=v_sb[:, kb * VW + kv * P: kb * VW + (kv + 1) * P],
                                rhs=pt[:, u * 512:(u + 1) * 512],
                                start=(kb == 0), stop=(kb == NKB - 1),
                            )
                    den = smallp.tile([64, 512], F32, tag="den")
                    nc.vector.reciprocal(den[:], av[64:128, :])
                    nc.vector.tensor_tensor(
                        out=hid[j][o:o + 64, qb * 512:(qb + 1) * 512],
                        in0=av[0:64, :], in1=den[:],
                        op=mybir.AluOpType.mult,
                    )

            # o-proj + bias + output DMA for this query block
            for tb in range(4):
                ot = outp.tile([P, D], F32, tag="osb")
                for jh in range(2):
                    ps = ppool.tile([P, 512], F32, tag="proj")
                    for ic in range(8):
                        nc.tensor.matmul(
                            ps[:],
                            lhsT=hid[ic][:, qb * 512 + tb * P: qb * 512 + (tb + 1) * P],
                            rhs=wot_sb[:, ic * D + jh * 512: ic * D + (jh + 1) * 512],
                            start=(ic == 0), stop=(ic == 7),
                        )
                    nc.vector.tensor_tensor(
                        out=ot[:, jh * 512:(jh + 1) * 512],
                        in0=ps[:], in1=bo_bc[:, jh * 512:(jh + 1) * 512],
                        op=mybir.AluOpType.add,
                    )
                nc.sync.dma_start(
                    out=out_p[qb * 512 + tb * P: qb * 512 + (tb + 1) * P, :],
                    in_=ot[:])

    _split_excess_waits(nc)
    return nc


def make_in_maps(x, wq, wk, wv, wo, bo):
    bf = ml_dtypes.bfloat16
    # device q block j (0..7), offset o in {0,64}: head kv=2*(j//4)+o//64,
    # g=j%4; original wq row for (kv, g, lane l) = kv*256 + g*64 + l
    dperm = np.empty(D, np.int64)
    for j in range(8):
        for o in (0, 1):
            kv = 2 * (j // 4) + o
            g = j % 4
            base = j * 128 + o * 64
            dperm[base:base + 64] = np.arange(kv * 256 + g * 64, kv * 256 + g * 64 + 64)
    wqt_h = np.ascontiguousarray(wq[dperm].T).astype(bf)   # [1024, 1024 dev dims]
    wkt_h = np.ascontiguousarray(wk.T).astype(bf)          # [1024, 256]
    wvt_h = np.ascontiguousarray(wv.T).astype(bf)
    wot_h = np.ascontiguousarray(wo.T[dperm]).astype(bf)   # [1024 dev dims, 1024]
    bo_h = bo.astype(np.float32).reshape(1, D)
    in_maps = []
    for c in range(NCORES):
        b, qh = c // 2, c % 2
        xb = x[b]
        if qh:
            xb = np.concatenate([xb[NQ:], xb[:NQ]], axis=0)  # own queries first
        in_maps.append({
            "xt": np.ascontiguousarray(xb.T).astype(bf),
            "wqt": wqt_h,
            "wkt": wkt_h,
            "wvt": wvt_h,
            "wot": wot_h,
            "bo_in": bo_h,
        })
    return in_maps


_CACHED_NC = None


def kernel(x, wq, wk, wv, wo, bo, _trace=False, _trace_kwargs=None):
    global _CACHED_NC
    from concourse.bass_utils import run_bass_kernel_spmd

    if _CACHED_NC is None:
        _CACHED_NC = build_nc()
    nc = _CACHED_NC

    in_maps = make_in_maps(
        np.asarray(x, np.float32), np.asarray(wq, np.float32),
        np.asarray(wk, np.float32), np.asarray(wv, np.float32),
        np.asarray(wo, np.float32), np.asarray(bo, np.float32))

    res = run_bass_kernel_spmd(
        nc, in_maps, core_ids=list(range(NCORES)),
        trace=_trace, **(_trace_kwargs or {}))

    out = np.empty((B, N, D), np.float32)
    for c in range(NCORES):
        b, qh = c // 2, c % 2
        out[b, qh * NQ:(qh + 1) * NQ] = res.results[c]["out_p"]
    if _trace:
        kernel._last_results = res
    return out


# revision 8
# speedup vs baseline: 1.5869x; 1.1823x over previous
"""GQA attention kernel for 8 TRN2 NeuronCores — sequence-split edition.

Problem: x[4,2048,1024], 16 Q heads / 4 KV heads, head_dim 64 (torch-Linear
style projections, softmax(QK^T/8)V, output projection + bias).

Sharding: core c handles (batch b = c//2, query-half qh = c%2): it computes
the FULL output rows for its 1024 query tokens (all 16 heads + o-proj), so
there is NO collective — each core DMAs its own [1024, 1024] f32 slab out.
K/V are computed for all 2048 keys on both cores of a pair (cheap).

The host permutes each core's token order so its own queries come first
(attention is key-order invariant), which keeps the SPMD program uniform.
Q-head order is permuted on the host so each head's 64 q-dims sit at the
same partition offset as its KV head's k-dims (QK lhsT/rhs share a base
partition): device q block j (0..7), offset o in {0,64} holds head
(kv = 2*(j//4) + o//64, g = j%4); wo^T rows are permuted identically.

Inside: q^T/k^T in [dim, token] layout so QK needs no transposes; S^T tiles
[keys=128, queries=512] are exp'd on ACT straight from PSUM; V is augmented
with 64 ones columns so the AV matmul also produces the softmax denominators
on partitions 64:128; normalization = reciprocal_approx_fast (single fast
custom-DVE op) + one DVE multiply writing hid^T in bf16. O-proj + bias-add +
output DMA stream per 512-query block, overlapped with the next block's
attention (ACT exp is the critical engine; everything else hides under it).
"""

import sys
import numpy as np
from contextlib import ExitStack

sys.path.insert(0, "/opt/trn_rl_repo")

import ml_dtypes

from concourse import bass, tile, mybir


# ---------------------------------------------------------------------------
# This walrus build encodes at most 1-2 sync waits per instruction; the stock
# TileContext tail drain packs one wait per live proc onto a single Drain and
# fails codegen ("Too many sync wait commands"). Spread the waits over SP nop
# carriers instead.
def _patched_drain_and_barrier(self, tick_clock, wait_clock):
    from concourse.vector_clock import ScopedClock, VectorClock

    nc = self.nc
    gc = tick_clock.global_clock
    n = len(gc)
    for proc in range(n):
        t = gc[proc]
        if t <= 0:
            continue
        carrier = nc.sync.nop(nofuse=True)
        req = VectorClock([t if i == proc else 0 for i in range(n)])
        wait_clock.add_sem_waits(carrier.ins, ScopedClock({None: req}))
    nc.sync.drain()
    nc.all_engine_barrier()
    assert self.sems is not None
    popped = nc._tile_sem_poison_stack.pop()
    assert popped is self._sem_poison
    nc.clear_and_free_semaphores(list(self.sems.allocated().values()))
    nc.all_engine_barrier()


tile.TileContext._drain_and_barrier = _patched_drain_and_barrier


def _split_excess_waits(nc, max_waits=1):
    """Hoist all but one sync wait per instruction onto dedicated
    EventSemaphore carriers placed immediately before it on the same engine
    (same blocking semantics, one wait per encoded instruction)."""
    n_new = 0
    for bb in nc.main_func.blocks:
        il = list(bb.instructions)
        out = []
        changed = False
        for ins in il:
            si = ins.sync_info
            if si is not None:
                w = list(si.on_wait)
                if len(w) > max_waits:
                    for extra in w[max_waits:]:
                        ev = mybir.InstEventSemaphore(
                            name=f"{ins.name}-wsp{n_new}", engine=ins.engine)
                        n_new += 1
                        ev.sync_info = type(si)(on_wait=[extra], on_update=[])
                        nc.register_instruction(ev, overwrite=True)
                        out.append(ev)
                    si.on_wait = w[:max_waits]
                    changed = True
            out.append(ins)
        if changed:
            bb.instructions = out
# ---------------------------------------------------------------------------

B, N, D = 4, 2048, 1024
DH = 64          # head dim
NQ = 1024        # queries per core
NCORES = 8
P = 128
SCALE = DH ** -0.5
BF16 = mybir.dt.bfloat16
F32 = mybir.dt.float32

NKB = N // P     # 16 key blocks of 128
NKC = D // P     # 8 contraction chunks of 128
KVD = 256        # total kv dims
VW = 512         # v chunk width per key block: 4 x [64 v | 64 ones]


def build_nc(st_bufs=2, av_bufs=2, pt_bufs=4):
    nc = bass.Bass(target_bir_lowering=False, debug=False, num_devices=NCORES)

    xt = nc.declare_dram_parameter("xt", [D, N], BF16, isOutput=False)
    wqt = nc.declare_dram_parameter("wqt", [D, D], BF16, isOutput=False)
    wkt = nc.declare_dram_parameter("wkt", [D, KVD], BF16, isOutput=False)
    wvt = nc.declare_dram_parameter("wvt", [D, KVD], BF16, isOutput=False)
    wot = nc.declare_dram_parameter("wot", [D, D], BF16, isOutput=False)
    bo_in = nc.declare_dram_parameter("bo_in", [1, D], F32, isOutput=False)
    out_p = nc.declare_dram_parameter("out_p", [NQ, D], F32, isOutput=True)

    with tile.TileContext(nc) as tc, ExitStack() as ctx:
        const = ctx.enter_context(tc.tile_pool(name="const", bufs=1))
        work = ctx.enter_context(tc.tile_pool(name="work", bufs=1))
        # one shared 4-buf PSUM pool for proj/o-proj/AV tiles (deep av
        # rotation so block N+4's wait is always satisfied) + 2x 2-bank st
        ppool = ctx.enter_context(tc.tile_pool(name="ppool", bufs=4, space="PSUM"))
        stpool = ctx.enter_context(tc.tile_pool(name="stp", bufs=st_bufs, space="PSUM"))
        ptpool = ctx.enter_context(tc.tile_pool(name="ptp", bufs=pt_bufs))
        smallp = ctx.enter_context(tc.tile_pool(name="smallp", bufs=3))
        outp = ctx.enter_context(tc.tile_pool(name="outp", bufs=3))

        # ---- load inputs (K weights + x first: K-proj unblocks earliest) --
        xt_sb = const.tile([P, NKC * N], BF16)
        wkt_sb = const.tile([P, NKC * KVD], BF16)
        wvt_sb = const.tile([P, NKC * KVD], BF16)
        wqt_sb = const.tile([P, NKC * D], BF16)
        wot_sb = const.tile([P, NKC * D], BF16)
        bo_row = const.tile([1, D], F32)
        for kc in range(NKC):
            nc.sync.dma_start(out=wkt_sb[:, kc * KVD:(kc + 1) * KVD],
                              in_=wkt[kc * P:(kc + 1) * P, :])
        for kc in range(NKC):
            nc.sync.dma_start(out=xt_sb[:, kc * N:(kc + 1) * N],
                              in_=xt[kc * P:(kc + 1) * P, :])
        for kc in range(NKC):
            nc.sync.dma_start(out=wvt_sb[:, kc * KVD:(kc + 1) * KVD],
                              in_=wvt[kc * P:(kc + 1) * P, :])
        for kc in range(NKC):
            nc.sync.dma_start(out=wqt_sb[:, kc * D:(kc + 1) * D],
                              in_=wqt[kc * P:(kc + 1) * P, :])
        for kc in range(NKC):
            nc.sync.dma_start(out=wot_sb[:, kc * D:(kc + 1) * D],
                              in_=wot[kc * P:(kc + 1) * P, :])
        nc.sync.dma_start(out=bo_row[:], in_=bo_in[0:1, :])
        ones_row = const.tile([1, P], F32)
        nc.vector.memset(ones_row[:], 1.0)

        # ---- projections -------------------------------------------------
        # k^T [256, 2048] as 2 partition-blocks (kv head kv at block kv//2,
        # partition offset (kv%2)*64)
        kt_sb = work.tile([P, 2 * N], BF16, tag="kt")
        for m2 in range(2):
            for nb in range(4):
                ps = ppool.tile([P, 512], F32, tag="proj")
                for kc in range(NKC):
                    nc.tensor.matmul(
                        ps[:],
                        lhsT=wkt_sb[:, kc * KVD + m2 * P: kc * KVD + (m2 + 1) * P],
                        rhs=xt_sb[:, kc * N + nb * 512: kc * N + (nb + 1) * 512],
                        start=(kc == 0), stop=(kc == NKC - 1),
                    )
                nc.vector.tensor_copy(kt_sb[:, m2 * N + nb * 512: m2 * N + (nb + 1) * 512], ps[:])

        # v natural [keys, vdim], augmented: per key block 4 x [64 v | 64 ones]
        v_sb = work.tile([P, NKB * VW], BF16, tag="v")
        nc.vector.memset(v_sb[:], 1.0)  # ones columns survive the copies
        for kb in range(NKB):
            ps = ppool.tile([P, KVD], F32, tag="proj")
            for kc in range(NKC):
                nc.tensor.matmul(
                    ps[:],
                    lhsT=xt_sb[:, kc * N + kb * P: kc * N + (kb + 1) * P],
                    rhs=wvt_sb[:, kc * KVD:(kc + 1) * KVD],
                    start=(kc == 0), stop=(kc == NKC - 1),
                )
            for kv in range(4):
                nc.vector.tensor_copy(
                    v_sb[:, kb * VW + kv * P: kb * VW + kv * P + 64],
                    ps[:, kv * 64:(kv + 1) * 64])

        # q^T [1024, 1024] in device head order, 8 partition-block tiles
        qt = []
        for j in range(8):
            qt_j = work.tile([P, NQ], BF16, tag=f"qt{j}")
            qt.append(qt_j)
            for q2 in range(2):
                ps = ppool.tile([P, 512], F32, tag="proj")
                for kc in range(NKC):
                    nc.tensor.matmul(
                        ps[:],
                        lhsT=wqt_sb[:, kc * D + j * P: kc * D + (j + 1) * P],
                        rhs=xt_sb[:, kc * N + q2 * 512: kc * N + (q2 + 1) * 512],
                        start=(kc == 0), stop=(kc == NKC - 1),
                    )
                nc.vector.tensor_copy(qt_j[:, q2 * 512:(q2 + 1) * 512], ps[:])

        # bias partition-broadcast via PE outer product (emitted after the
        # projections so the PE queue never stalls on the bo DMA up front)
        bo_bc = const.tile([P, D], F32)
        for jh in range(2):
            bps = ppool.tile([P, 512], F32, tag="proj")
            nc.tensor.matmul(bps[:], lhsT=ones_row[:, 0:P],
                             rhs=bo_row[:, jh * 512:(jh + 1) * 512],
                             start=True, stop=True)
            nc.vector.tensor_copy(bo_bc[:, jh * 512:(jh + 1) * 512], bps[:])

        # hidden^T [1024, 1024] bf16, device head order (matches wot rows)
        hid = []
        for j in range(8):
            hid_j = work.tile([P, NQ], BF16, tag=f"hid{j}")
            hid.append(hid_j)

        # ---- attention + streamed o-proj ---------------------------------
        for qb in range(2):  # 512-query blocks
            for j in range(8):
                for o in (0, 64):
                    kv = 2 * (j // 4) + o // 64
                    av = ppool.tile([P, 512], F32, tag="proj")
                    for kb2 in range(NKB // 2):  # key blocks in fused pairs
                        st = stpool.tile([P, 1024], F32, tag="st")  # 2 banks
                        for u in range(2):
                            kb = 2 * kb2 + u
                            nc.tensor.matmul(
                                st[:, u * 512:(u + 1) * 512],
                                lhsT=kt_sb[o:o + 64, (j // 4) * N + kb * P: (j // 4) * N + (kb + 1) * P],
                                rhs=qt[j][o:o + 64, qb * 512:(qb + 1) * 512],
                                start=True, stop=True,
                            )
                        # one ACT pass over both key blocks amortizes the
                        # ~290ns ACTIVATE pipeline overhead
                        pt = ptpool.tile([P, 1024], BF16, tag="pt")
                        nc.scalar.activation(pt[:], st[:],
                                             mybir.ActivationFunctionType.Exp,
                                             scale=SCALE)
                        for u in range(2):
                            kb = 2 * kb2 + u
                            nc.tensor.matmul(
                                av[:],
                                lhsT=v_sb[:, kb * VW + kv * P: kb * VW + (kv + 1) * P],
                                rhs=pt[:, u * 512:(u + 1) * 512],
                                start=(kb == 0), stop=(kb == NKB - 1),
                            )
                    den = smallp.tile([64, 512], F32, tag="den")
                    nc.vector.reciprocal(den[:], av[64:128, :])
                    nc.vector.tensor_tensor(
                        out=hid[j][o:o + 64, qb * 512:(qb + 1) * 512],
                        in0=av[0:64, :], in1=den[:],
                        op=mybir.AluOpType.mult,
                    )

            # o-proj + bias + output DMA for this query block
            for tb in range(4):
                ot = outp.tile([P, D], F32, tag="osb")
                for jh in range(2):
                    ps = ppool.tile([P, 512], F32, tag="proj")
                    for ic in range(8):
                        nc.tensor.matmul(
                            ps[:],
                            lhsT=hid[ic][:, qb * 512 + tb * P: qb * 512 + (tb + 1) * P],
                            rhs=wot_sb[:, ic * D + jh * 512: ic * D + (jh + 1) * 512],
                            start=(ic == 0), stop=(ic == 7),
                        )
                    nc.vector.tensor_tensor(
                        out=ot[:, jh * 512:(jh + 1) * 512],
                        in0=ps[:], in1=bo_bc[:, jh * 512:(jh + 1) * 512],
                        op=mybir.AluOpType.add,
                    )
                nc.sync.dma_start(
                    out=out_p[qb * 512 + tb * P: qb * 512 + (tb + 1) * P, :],
                    in_=ot[:])

    _split_excess_waits(nc)
    return nc


def make_in_maps(x, wq, wk, wv, wo, bo):
    bf = ml_dtypes.bfloat16
    # device q block j (0..7), offset o in {0,64}: head kv=2*(j//4)+o//64,
    # g=j%4; original wq row for (kv, g, lane l) = kv*256 + g*64 + l
    dperm = np.empty(D, np.int64)
    for j in range(8):
        for o in (0, 1):
            kv = 2 * (j // 4) + o
            g = j % 4
            base = j * 128 + o * 64
            dperm[base:base + 64] = np.arange(kv * 256 + g * 64, kv * 256 + g * 64 + 64)
    wqt_h = np.ascontiguousarray(wq[dperm].T).astype(bf)   # [1024, 1024 dev dims]
    wkt_h = np.ascontiguousarray(wk.T).astype(bf)          # [1024, 256]
    wvt_h = np.ascontiguousarray(wv.T).astype(bf)
    wot_h = np.ascontiguousarray(wo.T[dperm]).astype(bf)   # [1024 dev dims, 1024]
    bo_h = bo.astype(np.float32).reshape(1, D)
    in_maps = []
    for c in range(NCORES):
        b, qh = c // 2, c % 2
        xb = x[b]
        if qh:
            xb = np.concatenate([xb[NQ:], xb[:NQ]], axis=0)  # own queries first
        in_maps.append({
            "xt": np.ascontiguousarray(xb.T).astype(bf),
            "wqt": wqt_h,
            "wkt": wkt_h,
            "wvt": wvt_h,
            "wot": wot_h,
            "bo_in": bo_h,
        })
    return in_maps


_CACHED_NC = None


def kernel(x, wq, wk, wv, wo, bo, _trace=False, _trace_kwargs=None):
    global _CACHED_NC
    from concourse.bass_utils import run_bass_kernel_spmd

    if _CACHED_NC is None:
        _CACHED_NC = build_nc()
    nc = _CACHED_NC

    in_maps = make_in_maps(
        np.asarray(x, np.float32), np.asarray(wq, np.float32),
        np.asarray(wk, np.float32), np.asarray(wv, np.float32),
        np.asarray(wo, np.float32), np.asarray(bo, np.float32))

    res = run_bass_kernel_spmd(
        nc, in_maps, core_ids=list(range(NCORES)),
        trace=_trace, **(_trace_kwargs or {}))

    out = np.empty((B, N, D), np.float32)
    for c in range(NCORES):
        b, qh = c // 2, c % 2
        out[b, qh * NQ:(qh + 1) * NQ] = res.results[c]["out_p"]
    if _trace:
        kernel._last_results = res
    return out
